# revision 11
# baseline (speedup 1.0000x reference)
"""Trainium2 Bass kernel for nn_DynamicShortConvolution.

Reference computation (per token t, channel d):
    h    = silu(x @ w1)                       # [T, H]
    flat = h @ w2 + b2                        # [T, D*W]
    k    = flat.reshape(T, D, W)
    out[t, d] = silu(sum_w k[t, d, w] * x[t - (W-1) + w, d])

Sharding: 8 cores, each one (batch, half-of-T) shard of 2048 tokens plus a
3-token left halo.  Per-core tensors are TRANSPOSED ([D, T], channels on
SBUF partitions) so the conv's token shift is a free-dim offset and both
matmuls consume/produce the natural layouts (no on-device transposes).

Schedule (engineered from the measured TRN2 cost model):
 - PE warm-up matmuls at t=0 cover the initial DMA latency (HAM stays warm).
 - mm1 accumulates dt-OUTER so each x d-tile is consumed as its DMA lands;
   x tiles stream on the sync HWDGE ring in consumption order, w2 follows.
 - mm2 evac per (dt, 1024-token pair): DVE does the two odd-tap fused
   (k+b)*x stt ops (PSUM 1x mode) plus the two even-tap bf16 2x-mode
   multiplies; ACT does the two even-tap biased PSUM evacs + final silu;
   the 3 tap-sum adds run as SBUF->SBUF accumulate-DMAs (SWDGE CCE) in
   mode 'v2', or on GpSimd/DVE in mode 'v1'.
"""

import os

import numpy as np

# Problem constants (hardcoded per harness contract).
B, T, D, H, W = 4, 4096, 2048, 256, 4
HALO = W - 1
N_CORES = 8
TOK = (B * T) // N_CORES  # tokens per core = 2048
N_DT = D // 128           # 16 d-tiles
N_HC = H // 128           # 2 h-tiles
XSTRIDE = TOK + HALO + 1  # 2052, even keeps per-dtile 4B alignment
CH = 512                  # mm1 token chunk (one PSUM bank of fp32)
P = 1024                  # mm2 token pair width (two banks per tap)
N_WARM = 8                # PE warm-up matmuls (~3.4us at cold clock)


def _build_nc(mode="v2", out_f32=False):
    import concourse.bass as bass  # noqa: F401
    import concourse.bacc as bacc
    import concourse.mybir as mybir
    import concourse.tile as tile
    from concourse.ap import AP as BassAP

    f32 = mybir.dt.float32
    bf16 = mybir.dt.bfloat16
    AF = mybir.ActivationFunctionType
    ALU = mybir.AluOpType

    tok, h = TOK, H
    n_dt, n_hc = N_DT, N_HC

    nc = bacc.Bacc()

    # DRAM I/O (host-prepared layouts; see _prep_shards)
    xT = nc.declare_dram_parameter("xT", [n_dt, 128, XSTRIDE], bf16, isOutput=False)
    w1r = nc.declare_dram_parameter("w1r", [128, n_dt * h], bf16, isOutput=False)
    # w2r[dt, p, hc*512 + w*128 + i] = w2[hc*128+p, (dt*128+i)*W + w]
    w2r = nc.declare_dram_parameter("w2r", [n_dt, 128, n_hc * W * 128], bf16,
                                    isOutput=False)
    # b2r[p, dt*W + w] = b2[(dt*128+p)*W + w]
    b2r = nc.declare_dram_parameter("b2r", [128, n_dt * W], f32, isOutput=False)
    out_dt = f32 if out_f32 else bf16
    outT = nc.declare_dram_parameter("outT", [n_dt, 128, tok], out_dt, isOutput=True)

    with tile.TileContext(nc) as tc:
        with (
            tc.tile_pool(name="resident", bufs=1) as rpool,
            tc.tile_pool(name="work", bufs=3) as wpool,
            tc.tile_pool(name="psum", bufs=4, space="PSUM") as ppool,
        ):
            # ---- PE warm-up (covers initial DMA latency, warms HAM) ----
            warm_sb = rpool.tile([128, 640], bf16, tag="warm")
            nc.vector.memset(warm_sb[:], 0.0)
            for _ in range(N_WARM):
                wt = ppool.tile([128, P], f32, tag="ps")
                nc.tensor.matmul(wt[:, :CH], warm_sb[:, :128],
                                 warm_sb[:, 128:640], start=True, stop=True)

            # ---- resident tiles ----
            xT_sb = rpool.tile([128, n_dt * XSTRIDE], bf16, tag="xT")
            w1_sb = rpool.tile([128, n_dt * h], bf16, tag="w1")
            w2_sb = rpool.tile([128, n_dt * n_hc * W * 128], bf16, tag="w2")
            b2_sb = rpool.tile([128, n_dt * W], f32, tag="b2")
            hT_sb = rpool.tile([128, n_hc * tok], bf16, tag="hT")

            # DMA order = consumption order: w1, then x d-tiles, then w2.
            # b2 rides the scalar (ACT) HWDGE ring so it never queues x.
            nc.scalar.dma_start(b2_sb[:], b2r[:])
            nc.sync.dma_start(w1_sb[:], w1r[:])
            for dt in range(n_dt):
                nc.sync.dma_start(
                    xT_sb[:, dt * XSTRIDE:(dt + 1) * XSTRIDE], xT[dt])
            for dt in range(n_dt):
                nc.sync.dma_start(
                    w2_sb[:, dt * 1024:(dt + 1) * 1024], w2r[dt])

            def x_slice(dt, col, n):
                return xT_sb[:, dt * XSTRIDE + col: dt * XSTRIDE + col + n]

            # ---- mm1: hT = silu(w1.T @ xT), dt-OUTER accumulation ----
            # 8 groups (hc, tc) live in 4 [128,1024] psum tiles (2 banks
            # each, one bank per group) so each arriving x d-tile is
            # consumed immediately.
            ph = [ppool.tile([128, P], f32, tag="ps", name=f"ph{i}")
                  for i in range(4)]

            def ph_half(g):
                c = (g % 2) * CH
                return ph[g // 2][:, c:c + CH]

            for dt in range(n_dt):
                for hc in range(n_hc):
                    for tcb in range(4):
                        g = hc * 4 + tcb
                        nc.tensor.matmul(
                            ph_half(g),
                            w1_sb[:, dt * h + hc * 128: dt * h + hc * 128 + 128],
                            x_slice(dt, HALO + tcb * CH, CH),
                            start=(dt == 0), stop=(dt == n_dt - 1))
            for g in range(8):
                hc, tcb = g // 4, g % 4
                nc.scalar.activation(
                    hT_sb[:, hc * tok + tcb * CH: hc * tok + (tcb + 1) * CH],
                    ph_half(g), AF.Silu)

            # ---- mm2 + conv + silu, per (dt, 1024-token pair) ----
            # m_all layout is [m0 | m2 | m1 | m3] so the tap-sum tree is two
            # CONTIGUOUS SBUF->SBUF accumulate-DMAs (SWDGE CCE):
            #   AB: [m0|m2] += [m1|m3]   (one 512KB dma)
            #   C:  m0 += m2             (one 256KB dma)
            # The tail is software-pipelined so no engine FIFO ever waits on
            # DMA completions: C lags 2 iterations, silu/out-DMA lag 3.
            stageC = []   # [(m_all, dt, j0)] awaiting final accum
            stageS = []   # [(m_all, dt, j0)] C emitted, awaiting silu

            def emit_c():
                item = stageC.pop(0)
                pm = item[0]
                nc.gpsimd.dma_start(pm[:, 0:P], pm[:, P:2 * P],
                                    accum_op=ALU.add)
                stageS.append(item)

            def emit_silu():
                pm, pdt, pj0 = stageS.pop(0)
                ot = wpool.tile([128, P], out_dt, tag="ot", name="ot")
                nc.scalar.activation(ot[:], pm[:, 0:P], AF.Silu)
                nc.sync.dma_start(outT[pdt, :, pj0:pj0 + P], ot[:])

            for dt in range(n_dt):
                for pi in range(tok // P):
                    j0 = pi * P
                    kw = [ppool.tile([128, P], f32, tag="ps", name=f"kw{w}")
                          for w in range(W)]
                    for w in range(W):
                        wcol = dt * (n_hc * W * 128) + w * 128
                        for hc in range(n_hc):
                            for tcj in range(2):
                                nc.tensor.matmul(
                                    kw[w][:, tcj * CH:(tcj + 1) * CH],
                                    w2_sb[:, wcol + hc * W * 128:
                                          wcol + hc * W * 128 + 128],
                                    hT_sb[:, hc * tok + j0 + tcj * CH:
                                          hc * tok + j0 + (tcj + 1) * CH],
                                    start=(hc == 0), stop=(hc == n_hc - 1))
                    if len(stageC) >= 2:
                        emit_c()  # GpSimd FIFO head: deps long satisfied
                    bias = [b2_sb[:, dt * W + w: dt * W + w + 1]
                            for w in range(W)]
                    m_all = wpool.tile([128, 4 * P], bf16, tag="mall", bufs=5)
                    kb = wpool.tile([128, 2 * P], bf16, tag="kb")
                    # odd taps (2B-misaligned x windows): DVE fused
                    # evac+bias+mul, PSUM 1x mode
                    nc.vector.scalar_tensor_tensor(
                        m_all[:, 2 * P:3 * P], kw[1][:], bias[1],
                        x_slice(dt, j0 + 1, P), op0=ALU.add, op1=ALU.mult)
                    # even taps: ACT biased evac, then one paired DVE bf16
                    # 2x-mode mul over [kb0|kb2] (x windows 4B-aligned; the
                    # x operand is an overlapped-window 3D AP)
                    nc.scalar.activation(kb[:, 0:P], kw[0][:], AF.Identity,
                                         bias=bias[0])
                    nc.vector.scalar_tensor_tensor(
                        m_all[:, 3 * P:4 * P], kw[3][:], bias[3],
                        x_slice(dt, j0 + 3, P), op0=ALU.add, op1=ALU.mult)
                    nc.scalar.activation(kb[:, P:2 * P], kw[2][:], AF.Identity,
                                         bias=bias[2])
                    xs = x_slice(dt, j0, 1)
                    xpair = BassAP(tensor=xs.tensor, offset=xs.offset,
                                   ap=[list(xs.ap[0]), [2, 2], [1, P]])
                    nc.vector.tensor_mul(m_all[:, 0:2 * P], kb[:, 0:2 * P],
                                         xpair)
                    if len(stageS) >= 2:
                        emit_silu()
                    # accum tree in one contiguous dma: [m0|m2] += [m1|m3]
                    nc.gpsimd.dma_start(m_all[:, 0:2 * P], m_all[:, 2 * P:4 * P],
                                        accum_op=ALU.add)
                    stageC.append((m_all, dt, j0))
            while stageC:
                emit_c()
            while stageS:
                emit_silu()
    nc.compile()
    return nc


def _prep_shards(x, w1, w2, b2):
    """Host-side shard prep. Returns list of per-core in_maps."""
    import ml_dtypes
    bf16 = ml_dtypes.bfloat16

    n_dt, n_hc, tok = N_DT, N_HC, TOK
    b, t, d = x.shape
    shards_per_batch = (b * t // tok) // b

    w1_r = np.ascontiguousarray(
        w1.reshape(n_dt, 128, H).transpose(1, 0, 2).reshape(128, n_dt * H)
    ).astype(bf16)
    w2_r = np.ascontiguousarray(
        w2.reshape(n_hc, 128, n_dt, 128, W).transpose(2, 1, 0, 4, 3)
        .reshape(n_dt, 128, n_hc * W * 128)).astype(bf16)
    b2_r = np.ascontiguousarray(
        b2.reshape(n_dt, 128, W).transpose(1, 0, 2).reshape(128, n_dt * W)
    ).astype(np.float32)

    in_maps = []
    for core in range(N_CORES):
        bi, half = divmod(core, shards_per_batch)
        t0 = half * tok
        xh = np.zeros((tok + HALO, d), np.float32)
        lo = max(t0 - HALO, 0)
        xh[HALO - (t0 - lo):] = x[bi, lo: t0 + tok]
        xTc = np.zeros((n_dt, 128, XSTRIDE), bf16)
        xTc[:, :, : tok + HALO] = (
            xh.T.astype(bf16).reshape(n_dt, 128, tok + HALO))
        in_maps.append({"xT": xTc, "w1r": w1_r, "w2r": w2_r, "b2r": b2_r})
    return in_maps


_NC_CACHE = {}


def kernel(x, w1, w2, b2, trace=False):
    from concourse.bass_utils import run_bass_kernel_spmd

    mode = os.environ.get("KMODE", "v2")
    if mode not in _NC_CACHE:
        _NC_CACHE[mode] = _build_nc(mode=mode, out_f32=False)
    nc = _NC_CACHE[mode]

    in_maps = _prep_shards(
        np.asarray(x, np.float32), np.asarray(w1, np.float32),
        np.asarray(w2, np.float32), np.asarray(b2, np.float32))

    res = run_bass_kernel_spmd(nc, in_maps, core_ids=list(range(N_CORES)),
                               trace=trace)
    kernel.last_result = res

    shards_per_batch = (B * T // TOK) // B
    out = np.empty((B, T, D), np.float32)
    for core in range(N_CORES):
        bi, half = divmod(core, shards_per_batch)
        oT = res.results[core]["outT"]  # [n_dt, 128, tok]
        out[bi, half * TOK:(half + 1) * TOK] = (
            oT.reshape(D, TOK).T.astype(np.float32))
    return out
